# revision 1
# baseline (speedup 1.0000x reference)
"""Local window attention (7x7 windows, 8 heads, d=64) Trainium2 Bass kernel.

Full inputs in, full outputs out. Internally: data-parallel over batch across
8 NeuronCores (4 images per core). All shapes hardcoded per the problem spec:
  fmap (32, 56, 56, 256) f32, Wq (256,512), Wkv (256,1024), Wo (512,256), bo (256,)

Per-core dataflow (one "group" = 2 adjacent-y windows = 98 tokens, padded to
2x64 token slots on partitions so window w sits at partitions 64w..64w+48):
  f_raw [128,256]  <- DMA (2 windows)
  fT    [128,2,128](PE transpose)  c-on-partition
  qT,kT [128,4,98] = W.T @ fT      (4 n-chunks of 128, tokens compact 2x49)
  v     [128,512]  = f @ Wv        (token-padded rows)
  ST    [128,4,2,49] psum: per (chunk,hp,w): kT.T @ qT -> S^T [j,i]
  expS  = exp(SCALE * ST)          one ACT op
  out'  [128,2,2,65] psum x2: expS.T @ [v | ones] -> [i, 64+denom]
  out   [128,512] = out' * recip(denom)  (token-padded rows, head-major cols)
  outT  (PE transpose x4) -> final = outT.T @ Wo + bo -> DMA out
"""

from contextlib import ExitStack

import numpy as np

import concourse.bacc as bacc
import concourse.bass as bass
import concourse.tile as tile
from concourse import mybir
from concourse.masks import make_identity
from concourse.bass_utils import run_bass_kernel_spmd

P = 7
PP = 49          # tokens per window
H = 8            # heads
D = 64           # head dim
DIM = 256        # channels
INNER = 512      # h*d
SCALE = D ** -0.5
IMGS_PER_CORE = 4
NCORES = 8
X = 56
NW = X // P      # 8 windows per axis
FP32 = mybir.dt.float32


def build_bass(n_imgs=IMGS_PER_CORE):
    nc = bacc.Bacc("TRN2", target_bir_lowering=False, debug=False)

    fm = nc.dram_tensor("fmap", [n_imgs, X, X, DIM], FP32, kind="ExternalInput").ap()
    wq = nc.dram_tensor("Wq", [DIM, INNER], FP32, kind="ExternalInput").ap()
    wkv = nc.dram_tensor("Wkv", [DIM, 2 * INNER], FP32, kind="ExternalInput").ap()
    wo = nc.dram_tensor("Wo", [INNER, DIM], FP32, kind="ExternalInput").ap()
    bo = nc.dram_tensor("bo", [DIM], FP32, kind="ExternalInput").ap()
    out = nc.dram_tensor("out", [n_imgs, X, X, DIM], FP32, kind="ExternalOutput").ap()

    with tile.TileContext(nc) as tc:
        with ExitStack() as ctx:
            build_kernel(ctx, tc, out, fm, wq, wkv, wo, bo, n_imgs)
    nc.compile()
    return nc


def build_kernel(ctx, tc, out, fm, wq, wkv, wo, bo, n_imgs=IMGS_PER_CORE):
    nc = tc.nc
    consts = ctx.enter_context(tc.tile_pool(name="consts", bufs=1))
    sb = ctx.enter_context(tc.tile_pool(name="sb", bufs=3))
    ps = ctx.enter_context(tc.tile_pool(name="ps", bufs=8, space="PSUM"))

    # ---- constants ----
    ident = consts.tile([128, 128], FP32)
    make_identity(nc, ident[:])

    ones = consts.tile([128, 128], FP32)
    nc.gpsimd.memset(ones[:], 1.0)

    # weights, contraction dim (input channels) on partitions, chunked by 128
    wq_s = consts.tile([128, 2, INNER], FP32)   # [ck, kc, n]
    nc.sync.dma_start(out=wq_s[:], in_=wq.rearrange("(kc ck) n -> ck kc n", ck=128))
    wk_s = consts.tile([128, 2, INNER], FP32)
    nc.sync.dma_start(out=wk_s[:], in_=wkv[:, 0:INNER].rearrange("(kc ck) n -> ck kc n", ck=128))
    wv_s = consts.tile([128, 2, INNER], FP32)
    nc.sync.dma_start(out=wv_s[:], in_=wkv[:, INNER:2 * INNER].rearrange("(kc ck) n -> ck kc n", ck=128))
    wo_s = consts.tile([128, 4, DIM], FP32)     # [ck, kc, m]
    nc.sync.dma_start(out=wo_s[:], in_=wo.rearrange("(kc ck) m -> ck kc m", ck=128))
    bo_s = consts.tile([1, DIM], FP32)
    nc.sync.dma_start(out=bo_s[:], in_=bo[None, :])

    # ---- main loop: one group = 2 windows (same wx, adjacent wy) ----
    for img in range(n_imgs):
        for wx in range(NW):
            for u in range(NW // 2):
                group(nc, sb, ps, out, fm, wq_s, wk_s, wv_s, wo_s, bo_s, ident, ones,
                      img, wx, u)


def group(nc, sb, ps, out, fm, wq_s, wk_s, wv_s, wo_s, bo_s, ident, ones, img, wx, u):
    # 1. load 2 windows, token-padded: window w tokens at partitions 64w..64w+48
    f_raw = sb.tile([128, DIM], FP32, tag="f_raw")
    for w in range(2):
        wy = 2 * u + w
        for r in range(P):
            nc.sync.dma_start(
                out=f_raw[64 * w + P * r:64 * w + P * r + P, :],
                in_=fm[img, P * wx + r, P * wy:P * wy + P, :],
            )

    def ps_tile(shape):
        # uniform bank-sized psum slots; view-slice to the requested shape
        t = ps.tile([128, 512], FP32, tag="ps")
        n = int(np.prod(shape[1:]))
        v_ = t[:, 0:n]
        if len(shape) > 2:
            dims = " ".join(f"d{i}" for i in range(1, len(shape)))
            v_ = v_.rearrange(f"p ({dims}) -> p {dims}",
                              **{f"d{i}": shape[i] for i in range(1, len(shape) - 1)})
        return v_

    # 2-3. transpose -> fT [ck, kc, t]  (c on partitions, tokens padded on free)
    fT_ps = ps_tile([128, 2, 128])
    for kc in range(2):
        nc.tensor.transpose(fT_ps[:, kc, :], f_raw[:, 128 * kc:128 * kc + 128], ident[:])
    fT = sb.tile([128, 2, 128], FP32, tag="fT")
    nc.scalar.copy(fT[:], fT_ps[:])

    # 4-5. qT, kT [nc*128, 2x64 padded] = W.T @ fT
    qT_ps = ps_tile([128, 4, 128])
    kT_ps = ps_tile([128, 4, 128])
    for nk in range(4):
        for kc in range(2):
            nc.tensor.matmul(qT_ps[:, nk, :], wq_s[:, kc, 128 * nk:128 * nk + 128],
                             fT[:, kc, :], start=(kc == 0), stop=(kc == 1))
            nc.tensor.matmul(kT_ps[:, nk, :], wk_s[:, kc, 128 * nk:128 * nk + 128],
                             fT[:, kc, :], start=(kc == 0), stop=(kc == 1))
    # HW bug: matmul operands must start at partition 0 (high-half streaming
    # is broken), so split head-parities into base-0 tiles during the
    # mandatory psum->SBUF copies.
    qT = sb.tile([64, 4, 2, 128], FP32, tag="qT")   # [d, ch, hp, t]
    kT = sb.tile([64, 4, 2, 128], FP32, tag="kT")
    for hp in range(2):
        nc.vector.tensor_copy(qT[:, :, hp, :], qT_ps[64 * hp:64 * hp + 64, :, :])
        nc.scalar.copy(kT[:, :, hp, :], kT_ps[64 * hp:64 * hp + 64, :, :])

    # 6-7. v [t(padded), 512] = f @ Wv
    v_ps = ps_tile([128, INNER])
    for kc in range(2):
        nc.tensor.matmul(v_ps[:], fT[:, kc, :], wv_s[:, kc, :],
                         start=(kc == 0), stop=(kc == 1))
    v = sb.tile([64, 2, INNER], FP32, tag="v")      # [j, w, n]
    for w in range(2):
        nc.vector.tensor_copy(v[:, w, :], v_ps[64 * w:64 * w + 64, :])

    # 8-9. S^T then exp:  ST[j@64w, (ch, hp, i)]  (i padded to 64)
    # lhsT = kT slice with M=64 (incl. 15 pad cols) so psum rows are fully
    # written; pad lanes carry junk that is never consumed.
    st_ps = ps_tile([128, 4, 2, 64])
    for ch in range(4):
        for hp in range(2):
            for w in range(2):
                nc.tensor.matmul(
                    st_ps[64 * w:64 * w + 64, ch, hp, :],
                    kT[:, ch, hp, 64 * w:64 * w + 64],
                    qT[:, ch, hp, 64 * w:64 * w + 64],
                    tile_position=(0, 64 * w),
                )
    expS = sb.tile([64, 2, 4, 2, 64], FP32, tag="expS")  # [j, w, ch, hp, i]
    for w in range(2):
        nc.scalar.activation(expS[:, w, :, :, :], st_ps[64 * w:64 * w + 64, :, :, :],
                             mybir.ActivationFunctionType.Exp, scale=SCALE)

    # 10. out' = expS.T @ [v | 1]:   av[i@64w, (chL, hp, d|denom)]
    av_tiles = []
    for chpair in range(2):
        av = ps_tile([128, 2, 2, D + 1])
        av_tiles.append(av)
        for chL in range(2):
            ch = 2 * chpair + chL
            for hp in range(2):
                h = 2 * ch + hp
                for w in range(2):
                    # lhsT: K = 49 real keys (base 0), M = 64 (incl. pad
                    # queries so psum rows are fully written)
                    e = expS[0:PP, w, ch, hp, :]
                    nc.tensor.matmul(av[64 * w:64 * w + 64, chL, hp, 0:D],
                                     e, v[0:PP, w, D * h:D * h + D],
                                     tile_position=(0, 64 * w))
                    nc.tensor.matmul(av[64 * w:64 * w + 64, chL, hp, D:D + 1],
                                     e, ones[0:PP, 0:1],
                                     tile_position=(0, 64 * w))

    # 11-12. normalize: out_tok [t(padded), h*64+d]
    out_tok = sb.tile([128, INNER], FP32, tag="out_tok")
    for chpair in range(2):
        av = av_tiles[chpair]
        recd = sb.tile([128, 2, 2], FP32, tag="recd")
        nc.vector.reciprocal(recd[:], av[:, :, :, D])
        for chL in range(2):
            for hp in range(2):
                h = 2 * (2 * chpair + chL) + hp
                nc.vector.tensor_scalar(
                    out=out_tok[:, D * h:D * h + D],
                    in0=av[:, chL, hp, 0:D],
                    scalar1=recd[:, chL, hp:hp + 1],
                    scalar2=None,
                    op0=mybir.AluOpType.mult,
                )

    # 13-14. transpose out_tok -> outT [n, t(padded)]
    ot_ps = ps_tile([128, 4, 128])
    for nk in range(4):
        nc.tensor.transpose(ot_ps[:, nk, :], out_tok[:, 128 * nk:128 * nk + 128],
                            ident[:])
    outT = sb.tile([128, 4, 128], FP32, tag="outT")
    nc.scalar.copy(outT[:], ot_ps[:])

    # 15. final = outT.T @ Wo + bo   [t(padded), 256]
    fin_ps = ps_tile([128, DIM])
    for nk in range(4):
        nc.tensor.matmul(fin_ps[:], outT[:, nk, :], wo_s[:, nk, :],
                         start=(nk == 0), stop=False)
    nc.tensor.matmul(fin_ps[:], ones[0:1, 0:128], bo_s[:], start=False, stop=True)
    fin = sb.tile([128, DIM], FP32, tag="fin")
    nc.vector.tensor_copy(fin[:], fin_ps[:])

    # 16. store
    for w in range(2):
        wy = 2 * u + w
        for r in range(P):
            nc.sync.dma_start(
                out=out[img, P * wx + r, P * wy:P * wy + P, :],
                in_=fin[64 * w + P * r:64 * w + P * r + P, :],
            )


_CACHED = {}


def _get_nc():
    if "nc" not in _CACHED:
        _CACHED["nc"] = build_bass()
    return _CACHED["nc"]


def kernel(fmap, Wq, Wkv, Wo, bo, _trace=False, _trace_kwargs=None):
    fmap = np.ascontiguousarray(fmap, dtype=np.float32)
    nc = _get_nc()
    in_maps = []
    for c in range(NCORES):
        in_maps.append({
            "fmap": fmap[IMGS_PER_CORE * c:IMGS_PER_CORE * (c + 1)],
            "Wq": np.ascontiguousarray(Wq, dtype=np.float32),
            "Wkv": np.ascontiguousarray(Wkv, dtype=np.float32),
            "Wo": np.ascontiguousarray(Wo, dtype=np.float32),
            "bo": np.ascontiguousarray(bo, dtype=np.float32),
        })
    res = run_bass_kernel_spmd(nc, in_maps, core_ids=list(range(NCORES)),
                               trace=_trace, **(_trace_kwargs or {}))
    outs = [r["out"] for r in res.results]
    full = np.concatenate(outs, axis=0)
    if _trace:
        return full, res
    return full

